# revision 1
# baseline (speedup 1.0000x reference)
"""Trainium2 Bass kernel for nn_CorrelationImage.

reference:
    corr_b = sum(map1[b] * map2[b])            # dot over C*H*W per sample
    corr   = corr / ||corr||_2                 # L2 norm over the batch
    out    = map1 + map2 * (1 - corr)[:, None, None, None]

Sharding: data-parallel over batch B=64 across 8 cores (8 samples/core).
Per core:
  1. stream the 8 (map1, map2) sample pairs into SBUF (kept resident),
  2. per-sample dot: DVE multiply (free-dim) + ScalarE Copy-with-accum
     (free-dim reduce on ACT so DVE tracks the DMA loads) + a ones-matmul
     on PE (partition reduce, result replicated on all 128 partitions),
  3. AllReduce-add of the squared local dot vector (32 B) -> global
     sum-of-squares of the full 64-long corr vector on every core,
  4. s_i = 1 - c_i * rsqrt(ss) computed replicated, then in-place
     out_i = map2_i * s_i + map1_i (ScalarE scale, adds split across
     VectorE and GpSimd) streamed out per sample.
"""

import sys

import numpy as np

if "/opt/trn_rl_repo" not in sys.path:
    sys.path.insert(0, "/opt/trn_rl_repo")

B, C, H, W = 64, 64, 64, 64
N_CORES = 8
SPC = B // N_CORES  # samples per core
PART = 128
ELEMS = C * H * W  # 262144 per sample
FD = ELEMS // PART  # 2048 free-dim per sample tile

_cache = {}


def _build_nc(spc=SPC, fd=FD, n_cores=N_CORES, use_cc=True, cc_shared=True):
    from contextlib import ExitStack

    from concourse import bacc, tile, mybir

    f32 = mybir.dt.float32
    Alu = mybir.AluOpType
    Act = mybir.ActivationFunctionType

    nc = bacc.Bacc(
        "TRN2", target_bir_lowering=False, debug=False, num_devices=n_cores
    )
    m1d = nc.dram_tensor("map1", [spc, PART, fd], f32, kind="ExternalInput").ap()
    m2d = nc.dram_tensor("map2", [spc, PART, fd], f32, kind="ExternalInput").ap()
    outd = nc.dram_tensor("out", [spc, PART, fd], f32, kind="ExternalOutput").ap()

    with tile.TileContext(nc) as tc, ExitStack() as ctx:
        big = ctx.enter_context(tc.tile_pool(name="big", bufs=1))
        scratch = ctx.enter_context(tc.tile_pool(name="scratch", bufs=2))
        small = ctx.enter_context(tc.tile_pool(name="small", bufs=1))
        psum = ctx.enter_context(tc.tile_pool(name="psum", bufs=1, space="PSUM"))
        dram = ctx.enter_context(tc.tile_pool(name="dram", bufs=1, space="DRAM"))

        m1s = big.tile([PART, spc * fd], f32)
        m2s = big.tile([PART, spc * fd], f32)
        ones_t = small.tile([PART, PART], f32)
        nc.vector.memset(ones_t, 1.0)
        partials = small.tile([PART, spc], f32)
        # preload the sqrt table set off the critical path (Copy rides along)
        warm = small.tile([1, 1], f32)
        nc.vector.memset(warm, 1.0)
        nc.scalar.activation(out=warm, in_=warm, func=Act.Sqrt)

        for i in range(spc):
            nc.sync.dma_start(out=m1s[:, i * fd : (i + 1) * fd], in_=m1d[i])
            nc.sync.dma_start(out=m2s[:, i * fd : (i + 1) * fd], in_=m2d[i])

        # per-sample dot: process samples in PAIRS entirely on DVE — one
        # [128, 2*fd] multiply then one 3D tensor_reduce into two partials
        # columns; chained DVE ops issue back-to-back, so each pair costs
        # ~2x(2*fd) cycles with almost no per-op gap and tracks the loads
        npair = spc // 2
        for p in range(npair):
            prod = scratch.tile([PART, 2, fd], f32, name="prod")
            sl = slice(2 * p * fd, (2 * p + 2) * fd)
            nc.vector.tensor_mul(
                out=prod.rearrange("p a f -> p (a f)"),
                in0=m1s[:, sl],
                in1=m2s[:, sl],
            )
            nc.vector.tensor_reduce(
                out=partials[:, 2 * p : 2 * p + 2],
                in_=prod,
                axis=mybir.AxisListType.X,
                op=Alu.add,
            )

        # partition reduce; c_i replicated across all 128 partitions
        c8 = psum.tile([PART, spc], f32)
        nc.tensor.matmul(c8, ones_t, partials, start=True, stop=True)
        c8row = small.tile([1, spc], f32)
        nc.vector.tensor_copy(out=c8row, in_=c8[0:1, :])

        # squared local dots; AllReduce-add over cores then a free-dim
        # reduce gives the global sum of squares of the full corr vector
        csq = small.tile([1, spc], f32)
        nc.vector.tensor_mul(out=csq, in0=c8row, in1=c8row)
        cc_in = dram.tile([spc], f32)
        nc.sync.dma_start(out=cc_in[:], in_=csq[:])
        sqsum = small.tile([1, spc], f32)
        if use_cc:
            cc_out = dram.tile(
                [spc],
                f32,
                addr_space="Shared" if (cc_shared and n_cores > 4) else "Local",
            )
            nc.gpsimd.collective_compute(
                "AllReduce",
                Alu.add,
                replica_groups=[list(range(n_cores))],
                ins=[cc_in.opt()],
                outs=[cc_out.opt()],
            )
            nc.sync.dma_start(out=sqsum[:], in_=cc_out[:])
        else:
            # debug only: pretend every core holds the same 8 samples
            nc.vector.tensor_scalar_mul(out=sqsum, in0=csq, scalar1=float(n_cores))

        ss = small.tile([1, 1], f32)
        nc.vector.tensor_reduce(
            out=ss, in_=sqsum, axis=mybir.AxisListType.X, op=Alu.add
        )

        # replicate ss across partitions via a K=1 ones-matmul, then
        # s_i = 1 - c_i / sqrt(ss) computed on all partitions
        ssp = psum.tile([PART, 1], f32)
        nc.tensor.matmul(ssp, ones_t[0:1, :], ss, start=True, stop=True)
        normb = small.tile([PART, 1], f32)
        nc.scalar.activation(out=normb, in_=ssp, func=Act.Sqrt)
        inv = small.tile([PART, 1], f32)
        nc.vector.reciprocal(out=inv, in_=normb)
        ninv = small.tile([PART, 1], f32)
        nc.vector.tensor_scalar_mul(out=ninv, in0=inv, scalar1=-1.0)
        s8 = small.tile([PART, spc], f32)
        nc.vector.tensor_scalar(
            out=s8,
            in0=c8,
            scalar1=ninv,
            scalar2=1.0,
            op0=Alu.mult,
            op1=Alu.add,
        )

        # out_i = map2_i * s_i + map1_i, fully in place in the map2 buffer;
        # ScalarE does the per-sample scale (clean 2 us pace), the adds run
        # on DVE over PAIRS of adjacent samples (halves the per-op drain
        # tax), stores stream out per sample
        for i in range(spc):
            sl = slice(i * fd, (i + 1) * fd)
            nc.scalar.activation(
                out=m2s[:, sl],
                in_=m2s[:, sl],
                func=Act.Copy,
                scale=s8[:, i : i + 1],
            )
            if i % 2 == 1:
                psl = slice((i - 1) * fd, (i + 1) * fd)
                nc.vector.tensor_add(
                    out=m2s[:, psl], in0=m2s[:, psl], in1=m1s[:, psl]
                )
                nc.sync.dma_start(out=outd[i - 1], in_=m2s[:, (i - 1) * fd : i * fd])
                nc.sync.dma_start(out=outd[i], in_=m2s[:, sl])
        if spc % 2 == 1:
            sl = slice((spc - 1) * fd, spc * fd)
            nc.vector.tensor_add(out=m2s[:, sl], in0=m2s[:, sl], in1=m1s[:, sl])
            nc.sync.dma_start(out=outd[spc - 1], in_=m2s[:, sl])

    nc.compile()
    return nc


def _get_nc():
    if "nc" not in _cache:
        _cache["nc"] = _build_nc()
    return _cache["nc"]


def kernel(map1, map2):
    from concourse.bass_utils import run_bass_kernel_spmd

    nc = _get_nc()
    m1 = np.ascontiguousarray(np.asarray(map1, dtype=np.float32)).reshape(
        N_CORES, SPC, PART, FD
    )
    m2 = np.ascontiguousarray(np.asarray(map2, dtype=np.float32)).reshape(
        N_CORES, SPC, PART, FD
    )
    in_maps = [{"map1": m1[c], "map2": m2[c]} for c in range(N_CORES)]
    res = run_bass_kernel_spmd(nc, in_maps, list(range(N_CORES)))
    out = np.concatenate(
        [res.results[c]["out"].reshape(SPC, C, H, W) for c in range(N_CORES)],
        axis=0,
    )
    return out

